# revision 18
# baseline (speedup 1.0000x reference)
"""BigBird block-sparse attention on 8 Trainium2 NeuronCores.

Sharding: core = (batch b, head-group hg): b = core//4, hg = core%4.
Each core computes, for its batch and its 4 heads, all in f16:
  qT/kT = (W{q,k}[hs] @ x.T)            [256, 2048]  (q pre-scaled by 1/8)
  v     = x @ Wv[hs].T                  [2048, 256]  (natural layout + ones col)
  Scores per PAIR of key blocks (2g, 2g+1), parity-exact:
    'B' piece: q-blocks kept by BOTH -> one [64,128] stationary matmul
               writes both blocks' score rows [128, cols]
    'E'/'O' piece: q-blocks kept by only the even/odd member -> 64-row matmul
               into the top/bottom half (E and O co-run on distinct array
               column groups)
  expS  = exp(S.T) packed in PSUM fills, evicted to SBUF (f16). No masking:
          packing is exact, holes never materialize.
  outT  = [v|1].T @ expS  [65, 2048]: K=128 matmuls for B pieces, K=64
          half-array matmuls for E/O (serialized against each other per
          PSUM accumulate-race rules)
  attnT = outT[0:64] * (1/outT[64]) per head  -> [256, 2048] f16
  out  += attnT.T @ Wo[:, hs].T         [2048, 1024] f16 partial per head grp
Host gathers: out[b] = f32 sum over the 4 head-group cores of that batch.
"""

import os
import sys
import types

import numpy as np

_B, _L, _D = 2, 2048, 1024
_H, _HD, _BLK = 16, 64, 64
_NB = _L // _BLK  # 32
_NG = _NB // 2    # 16 key-block pairs
_NCORES = 8
_HPC = 4  # heads per core
_FILLW = 1024  # packed-psum fill width (2 PSUM banks, f32)

_cache = {}


# --------------------------------------------------------------------------
# host-side plan: derive the parity-exact pair-block structure once
# --------------------------------------------------------------------------
def _runs_of(qs):
    runs = []
    s = p = qs[0]
    for x in qs[1:]:
        if x == p + 1:
            p = x
        else:
            runs.append((s, p))
            s = p = x
    runs.append((s, p))
    return runs


def _build_plan(bm):
    """bm: [NB, NB] bool block mask (bm[q, j] = q-block attends key-block j).

    Pieces are placed on a global column strip (fill fi covers cols
    [fi*FILLW, (fi+1)*FILLW)); 'B' (both-parity) pieces are packed first
    across all groups (advancing both row-half cursors in lockstep, no
    waste), then 'E'/'O' pieces pack their half-strips independently.
    """
    pieces = []   # dicts: cls('B','G','E','O'), g, q0, n, pos (global col)

    ce = [0]  # even-half cursor (global strip cols)
    co = [0]

    def place(cls, g, q0, n):
        while n > 0:
            if cls in ("B", "G"):
                c = max(ce[0], co[0])
            else:
                c = (ce if cls == "E" else co)[0]
            room = (512 - c % 512) // _BLK
            if room == 0:
                c = (c // 512 + 1) * 512
                room = 8
            if cls == "G":
                take = n  # 2 strided blocks, 128 cols, never split
                if room < 2:
                    c = (c // 512 + 1) * 512
            else:
                take = min(n, room, 8)
            pieces.append(dict(cls=cls, g=g, q0=q0, n=take, pos=c))
            c += take * _BLK
            if cls in ("B", "G"):
                ce[0] = co[0] = c
            elif cls == "E":
                ce[0] = c
            else:
                co[0] = c
            q0 += take
            n -= take

    groups = []
    for g in range(_NG):
        A = set(np.nonzero(bm[:, 2 * g])[0].tolist())
        Bk = set(np.nonzero(bm[:, 2 * g + 1])[0].tolist())
        I = sorted(A & Bk)
        EA = sorted(A - Bk)
        EO = sorted(Bk - A)
        groups.append((I, EA, EO))

    # pass 1: both-parity pieces. All 'G' (strided {0,31}) pieces first:
    # they are 128 cols each so the cursor never skips a bank, which keeps
    # both half-strips gap-free (every PSUM byte the exp reads is written).
    binfo = []
    for g, (I, EA, EO) in enumerate(groups):
        ri = _runs_of(I)
        g2 = (len(ri) >= 2 and ri[0] == (0, 0) and ri[-1] == (31, 31))
        if g2:
            ri = ri[1:-1]
            place("G", g, 0, 2)
        binfo.append(ri)
    for g, ri in enumerate(binfo):
        for (s, e) in ri:
            place("B", g, s, e - s + 1)
    # pass 2: per-parity exclusive pieces
    for g, (I, EA, EO) in enumerate(groups):
        for (s, e) in (_runs_of(EA) if EA else []):
            place("E", g, s, e - s + 1)
        for (s, e) in (_runs_of(EO) if EO else []):
            place("O", g, s, e - s + 1)

    n_fills = (max(ce[0], co[0]) + _FILLW - 1) // _FILLW

    # pad the half-strip tails with zero pieces so exp never reads stale PSUM
    for cls, cur in (("E", ce), ("O", co)):
        while cur[0] < n_fills * _FILLW:
            span = min(512 - cur[0] % 512, n_fills * _FILLW - cur[0])
            pieces.append(dict(cls="Z" + cls, g=0, q0=0, n=span // _BLK,
                               pos=cur[0]))
            cur[0] += span

    # bucket pieces into fills, compute in-fill offsets and AV sub-pieces
    fills = [[] for _ in range(n_fills)]
    for pc in pieces:
        fi = pc["pos"] // _FILLW
        off = pc["pos"] % _FILLW
        if pc["cls"] == "G":
            avs = [(0, 1, off), (31, 1, off + _BLK)]
        elif pc["cls"] in ("ZE", "ZO"):
            avs = []
        else:
            avs = []
            a = pc["q0"]
            while a < pc["q0"] + pc["n"]:
                lim = min(pc["q0"] + pc["n"], ((a // 8) + 1) * 8)
                avs.append((a, lim - a, off + (a - pc["q0"]) * _BLK))
                a = lim
        fills[fi].append(dict(cls=pc["cls"], g=pc["g"], q0=pc["q0"],
                              n=pc["n"], off=off, avs=avs))

    # emission orders per fill:
    #   scores: B/G first, then E and O interleaved (col-group co-run)
    #   AV: two passes (v1-proven): E+B interleaved with fills, then a
    #   single O pass at the end of the head behind one serializing dep
    #   (E rows 0-63 and O rows 64-127 would otherwise race on the outT
    #   accumulate via PE row-group concurrency)
    for fi in range(n_fills):
        bs = [p for p in fills[fi] if p["cls"] in ("B", "G")]
        es = [p for p in fills[fi] if p["cls"] in ("E", "ZE")]
        os_ = [p for p in fills[fi] if p["cls"] in ("O", "ZO")]
        inter = []
        for i in range(max(len(es), len(os_))):
            if i < len(es):
                inter.append(es[i])
            if i < len(os_):
                inter.append(os_[i])
        fills[fi] = dict(scores=bs + inter,
                         av=[p for p in es if p["cls"] == "E"] + bs,
                         av_o=[p for p in os_ if p["cls"] == "O"])

    # AV stop flags per outT PSUM bank (512-col granularity), in the exact
    # AV emission order (all fills' E+B passes, then all O passes)
    exec_order = []
    for fi in range(n_fills):
        for pi, pc in enumerate(fills[fi]["av"]):
            for (a, nb, off) in pc["avs"]:
                exec_order.append((fi, 0, pi, a))
    for fi in range(n_fills):
        for pi, pc in enumerate(fills[fi]["av_o"]):
            for (a, nb, off) in pc["avs"]:
                exec_order.append((fi, 1, pi, a))
    av_flags = {}
    last_piece = {}
    for key in exec_order:
        last_piece[key[3] // 8] = key
    for key in exec_order:
        av_flags[key] = last_piece[key[3] // 8] == key

    return dict(fills=fills, n_fills=n_fills, av_flags=av_flags)


# --------------------------------------------------------------------------
# numpy simulator of the planned pipeline (used by test_plan.py)
# --------------------------------------------------------------------------
def _sim_plan(plan, q, k, v):
    """q, k, v: [L, 64] f32 (q pre-scaled by 1/8). Returns attn out [L, 64]."""
    nf = plan["n_fills"]
    expS = np.zeros((128, nf * _FILLW), np.float32)
    outT = np.zeros((65, _L), np.float64)
    rows = dict(B=(0, 128), G=(0, 128), E=(0, 64), O=(64, 128))
    for fi in range(nf):
        fill = plan["fills"][fi]
        ps = np.zeros((128, _FILLW), np.float32)
        for pc in fill["scores"]:
            if pc["cls"] in ("ZE", "ZO"):
                continue
            g, q0, n, off = pc["g"], pc["q0"], pc["n"], pc["off"]
            r0, r1 = rows[pc["cls"]]
            kp = k[g * 128 + r0: g * 128 + r1]
            if pc["cls"] == "G":
                qsel = np.concatenate([q[0:64], q[31 * 64:32 * 64]], axis=0)
            else:
                qsel = q[q0 * 64:(q0 + n) * 64]
            ps[r0:r1, off:off + qsel.shape[0]] = kp @ qsel.T
        expS[:, fi * _FILLW:(fi + 1) * _FILLW] = np.exp(ps)
        for pc in fill["av"] + fill["av_o"]:
            g = pc["g"]
            r0, r1 = rows[pc["cls"]]
            vj = np.concatenate(
                [v[g * 128 + r0: g * 128 + r1],
                 np.ones((r1 - r0, 1), np.float32)], axis=1)
            for (a, n, off) in pc["avs"]:
                e = expS[r0:r1, fi * _FILLW + off: fi * _FILLW + off + n * 64]
                outT[:, a * 64:(a + n) * 64] += vj.T @ e
    return (outT[0:64] / outT[64]).T


# --------------------------------------------------------------------------
# bass kernel build
# --------------------------------------------------------------------------
def _patch_ldw_opt():
    """Re-enable walrus's LDWEIGHTS dedup pass (concourse pins it off)."""
    if os.environ.get("BIGBIRD_LDW_OPT", "0") != "1":
        return
    import concourse.bass_utils as bu
    if getattr(bu, "_bigbird_ldw_patched", False):
        return
    orig = bu.run_command

    def run_command(cmd, *a, **k):
        cmd = [c.replace("--enable-ldw-opt=false", "--enable-ldw-opt=true")
               if isinstance(c, str) else c for c in cmd]
        return orig(cmd, *a, **k)

    bu.run_command = run_command
    bu._bigbird_ldw_patched = True


def _build_nc(plan):
    _patch_ldw_opt()
    import concourse.bacc as bacc
    from concourse.bass import _add_dep_helper as _add_dep
    import concourse.mybir as mybir
    from concourse.tile import TileContext

    f32r = mybir.dt.float32r
    f32 = mybir.dt.float32
    f16 = mybir.dt.float16
    EXP = mybir.ActivationFunctionType.Exp
    COPY = mybir.ActivationFunctionType.Copy

    NKC = _D // 128   # 8 contraction chunks
    NM = _L // 128    # 16 L tiles
    nf = plan["n_fills"]

    nc = bacc.Bacc(None, target_bir_lowering=False)

    xt = nc.dram_tensor("xt", [_D, _L], f16, kind="ExternalInput")
    wq = nc.dram_tensor("wq", [_D, 256], f16, kind="ExternalInput")
    wk = nc.dram_tensor("wk", [_D, 256], f16, kind="ExternalInput")
    wv = nc.dram_tensor("wv", [_D, 256], f16, kind="ExternalInput")
    wo = nc.dram_tensor("wo", [256, _D], f16, kind="ExternalInput")
    out = nc.dram_tensor("out", [_L, _D], f16, kind="ExternalOutput")

    with TileContext(nc) as tc:
        with tc.tile_pool(name="persist_sb", bufs=1) as psb:
            # ---- persistent SBUF ----
            wob = psb.tile([128, 2 * _D], f16, name="wob", tag="wob")
            qT = [psb.tile([128, _L], f16, name=f"qT{c}", tag=f"qT{c}")
                  for c in range(2)]
            kT = [psb.tile([128, _L], f16, name=f"kT{c}", tag=f"kT{c}")
                  for c in range(2)]
            # v' packed: per head 16 pairs x 65 cols (64 v + ones)
            vp = psb.tile([128, _HPC * 16 * 65], f16, name="vp", tag="vp")
            attnT = [psb.tile([128, _L], f16, name=f"attnT{c}", tag=f"attnT{c}")
                     for c in range(2)]
            ones_sb = psb.tile([1, 64], f16, name="ones_sb", tag="ones_sb")
            z65 = psb.tile([1, 65], f16, name="z65", tag="z65")
            z512 = psb.tile([1, 512], f16, name="z512", tag="z512")
            nc.vector.memset(ones_sb[:], 1.0)
            nc.vector.memset(z65[:], 0.0)
            nc.vector.memset(z512[:], 0.0)
            # ones columns of v'
            for h in range(_HPC):
                nc.vector.memset(
                    vp[:, h * 1040 + 64: h * 1040 + 16 * 65: 65], 1.0)

            with tc.tile_pool(name="load_sb", bufs=1) as lsb:
                # ---- input DMA (merged descriptors) ----
                xtb = lsb.tile([128, NKC * _L], f16, name="xtb", tag="xtb")
                wqb = lsb.tile([128, NKC * 256], f16, name="wqb", tag="wqb")
                wkb = lsb.tile([128, NKC * 256], f16, name="wkb", tag="wkb")
                wvb = lsb.tile([128, NKC * 256], f16, name="wvb", tag="wvb")
                for kc in range(NKC):
                    nc.sync.dma_start(wqb[:, kc * 256:(kc + 1) * 256],
                                      wq[kc * 128:(kc + 1) * 128, :])
                    nc.sync.dma_start(wkb[:, kc * 256:(kc + 1) * 256],
                                      wk[kc * 128:(kc + 1) * 128, :])
                    nc.sync.dma_start(xtb[:, kc * _L:(kc + 1) * _L],
                                      xt[kc * 128:(kc + 1) * 128, :])
                    nc.sync.dma_start(wvb[:, kc * 256:(kc + 1) * 256],
                                      wv[kc * 128:(kc + 1) * 128, :])
                for c in range(2):
                    nc.sync.dma_start(wob[:, c * _D:(c + 1) * _D],
                                      wo[c * 128:(c + 1) * 128, :])

                # ---- projections (Q, K) ----
                with tc.tile_pool(name="proj_ps", bufs=1, space="PSUM") as pps:
                    for (wb, dst) in ((wqb, qT), (wkb, kT)):
                        for half in range(2):  # L halves for earlier PE start
                            pt = [pps.tile([128, 512], f32, name=f"pp{mc}{nwi}",
                                           tag=f"pp{mc}{nwi}")
                                  for mc in range(2) for nwi in range(2)]
                            for kc in range(NKC):
                                for mc in range(2):
                                    for nwi in range(2):
                                        nw = half * 2 + nwi
                                        nc.tensor.matmul(
                                            pt[mc * 2 + nwi][:],
                                            wb[:, kc * 256 + mc * 128:
                                               kc * 256 + (mc + 1) * 128],
                                            xtb[:, kc * _L + nw * 512:
                                                kc * _L + (nw + 1) * 512],
                                            start=(kc == 0), stop=(kc == NKC - 1))
                            for mc in range(2):
                                for nwi in range(2):
                                    nw = half * 2 + nwi
                                    if nwi == 0:
                                        nc.scalar.activation(
                                            dst[mc][:, nw * 512:(nw + 1) * 512],
                                            pt[mc * 2 + nwi][:], COPY)
                                    else:
                                        nc.vector.tensor_copy(
                                            dst[mc][:, nw * 512:(nw + 1) * 512],
                                            pt[mc * 2 + nwi][:])
                    # ---- V projection (natural layout) ----
                    for m in range(NM):
                        pv = pps.tile([128, 256], f32, name="pv", tag="pv", bufs=3)
                        for kc in range(NKC):
                            nc.tensor.matmul(
                                pv[:],
                                xtb[:, kc * _L + m * 128: kc * _L + (m + 1) * 128],
                                wvb[:, kc * 256:(kc + 1) * 256],
                                start=(kc == 0), stop=(kc == NKC - 1))
                        # scatter 4 heads into v' tile (pair index = m)
                        vdst = vp[:].rearrange("p (h c) -> p h c", c=1040)
                        vsrc = pv[:].rearrange("p (h d) -> p h d", d=64)
                        nc.vector.tensor_copy(
                            vdst[:, :, m * 65: m * 65 + 64], vsrc[:, :, :])

            _rows = dict(B=(0, 128), G=(0, 128), E=(0, 64), O=(64, 128),
                         ZE=(0, 64), ZO=(64, 128))
            with tc.tile_pool(name="att_sb", bufs=1) as asb:
                # ---- attention per head ----
                with tc.tile_pool(name="att_ps", bufs=1, space="PSUM") as aps:
                    norm_q = []  # deferred per-head normalization tails

                    def emit_norm_tail():
                        c, pb, oT_sb, rec = norm_q.pop(0)
                        bc = [aps.tile([128, _FILLW], f32, name="sfill",
                                       tag="sfill", bufs=2) for _ in range(2)]
                        for w in range(4):
                            nc.tensor.matmul(
                                bc[w // 2][0:64, (w % 2) * 512:(w % 2 + 1) * 512],
                                ones_sb[:],
                                rec[:, w * 512:(w + 1) * 512],
                                start=True, stop=True)
                        for w in range(4):
                            nc.vector.tensor_mul(
                                attnT[c][pb:pb + 64, w * 512:(w + 1) * 512],
                                oT_sb[0:64, w * 512:(w + 1) * 512],
                                bc[w // 2][0:64, (w % 2) * 512:(w % 2 + 1) * 512])

                    for h in range(_HPC):
                        c, pb = h // 2, (h % 2) * 64
                        expS = asb.tile([128, nf * _FILLW], f16, name="expS",
                                        tag="expS", bufs=2)
                        outT = aps.tile([128, _L], f32, name="outT", tag="outT")
                        # start each outT bank's accumulation group with a
                        # full-bank zeroing matmul; AV pieces then accumulate
                        for bank in range(4):
                            nc.tensor.matmul(
                                outT[0:65, bank * 512:(bank + 1) * 512],
                                z65[:], z512[:],
                                start=True, stop=False,
                                tile_position=(0, 0))
                        # rows 0-63: attn out, row 64: sums
                        oT_sb = asb.tile([65, _L], f32r, name="oT_sb",
                                         tag="oT_sb", bufs=2)
                        rec = asb.tile([1, _L], f16, name="rec",
                                       tag="rec", bufs=2)
                        last_eb_mm = None
                        for fi in range(nf):
                            fill = plan["fills"][fi]
                            ps = aps.tile([128, _FILLW], f32, name="sfill",
                                          tag="sfill", bufs=2)
                            for pc in fill["scores"]:
                                g, q0, n, off = (pc["g"], pc["q0"], pc["n"],
                                                 pc["off"])
                                r0, r1 = _rows[pc["cls"]]
                                if pc["cls"] in ("ZE", "ZO"):
                                    # tail pad: write zeros so exp never
                                    # reads stale PSUM
                                    nc.tensor.matmul(
                                        ps[r0:r1, off:off + n * 64],
                                        z65[:, 0:r1 - r0],
                                        z512[:, 0:n * 64],
                                        start=True, stop=True,
                                        tile_position=(0, r0))
                                    continue
                                if pc["cls"] == "G":
                                    rhs = qT[c][pb:pb + 64, :].rearrange(
                                        "p (a b) -> p a b", b=64)[:, 0:32:31, :]
                                else:
                                    rhs = qT[c][pb:pb + 64,
                                                q0 * 64:(q0 + n) * 64]
                                nc.tensor.matmul(
                                    ps[r0:r1, off:off + n * 64],
                                    kT[c][pb:pb + 64,
                                          g * 128 + r0: g * 128 + r1],
                                    rhs,
                                    start=True, stop=True,
                                    tile_position=(pb, r0))
                            fsl = slice(fi * _FILLW, (fi + 1) * _FILLW)
                            nc.scalar.activation(expS[:, fsl], ps[:], EXP)
                            for pi, pc in enumerate(fill["av"]):
                                g = pc["g"]
                                r0, r1 = _rows[pc["cls"]]
                                for (a, n, off) in pc["avs"]:
                                    sp = plan["av_flags"][(fi, 0, pi, a)]
                                    last_eb_mm = nc.tensor.matmul(
                                        outT[0:65, a * 64:(a + n) * 64],
                                        vp[r0:r1, h * 1040 + g * 65:
                                           h * 1040 + g * 65 + 65],
                                        expS[r0:r1, fi * _FILLW + off:
                                             fi * _FILLW + off + n * 64],
                                        start=False, stop=sp,
                                        tile_position=(r0, 0))
                        # O pass: rows 64-127; must not overlap the E pass in
                        # the PE (same outT accumulate targets), so the first
                        # O matmul explicitly syncs on the last E/B one
                        first_o = True
                        for fi in range(nf):
                            for pi, pc in enumerate(plan["fills"][fi]["av_o"]):
                                g = pc["g"]
                                for (a, n, off) in pc["avs"]:
                                    sp = plan["av_flags"][(fi, 1, pi, a)]
                                    mm = nc.tensor.matmul(
                                        outT[0:65, a * 64:(a + n) * 64],
                                        vp[64:128, h * 1040 + g * 65:
                                           h * 1040 + g * 65 + 65],
                                        expS[64:128, fi * _FILLW + off:
                                             fi * _FILLW + off + n * 64],
                                        start=False, stop=sp,
                                        tile_position=(64, 0))
                                    if first_o and last_eb_mm is not None:
                                        _add_dep(mm.ins, last_eb_mm.ins,
                                                 sync=True,
                                                 reason="PSUM row-group serialize")
                                        first_o = False
                        # evict outT fast (split ACT/DVE), then free it for
                        # the next head; the recip chain runs in parallel
                        nc.scalar.activation(oT_sb[0:65, 0:1024],
                                             outT[0:65, 0:1024], COPY)
                        nc.vector.tensor_copy(oT_sb[0:65, 1024:2048],
                                              outT[0:65, 1024:2048])
                        recT = asb.tile([128, 16], f32r, name="recT",
                                        tag="recT", bufs=2)
                        recT2 = asb.tile([128, 16], f16, name="recT2",
                                         tag="recT2", bufs=2)
                        nc.sync.dma_start(recT[:], oT_sb[64:65, :])
                        with nc.allow_low_precision("fp16 softmax denominators"):
                            nc.vector.reciprocal(recT2[:], recT[:])
                        nc.sync.dma_start(rec[:], recT2[:])
                        norm_q.append((c, pb, oT_sb, rec))
                        if h > 0:
                            emit_norm_tail()
                    emit_norm_tail()

                # ---- output projection ----
                with tc.tile_pool(name="o_ps", bufs=4, space="PSUM") as ops:
                    for m in range(NM):
                        po = [ops.tile([128, 512], f32, name="po", tag=f"po{nw}")
                              for nw in range(2)]
                        for nw in range(2):
                            for c in range(2):
                                nc.tensor.matmul(
                                    po[nw][:],
                                    attnT[c][:, m * 128:(m + 1) * 128],
                                    wob[:, c * _D + nw * 512:
                                        c * _D + (nw + 1) * 512],
                                    start=(c == 0), stop=(c == 1))
                        ob = asb.tile([128, _D], f16, name="ob", tag="ob", bufs=3)
                        for nw in range(2):
                            if nw == 0:
                                nc.scalar.activation(
                                    ob[:, nw * 512:(nw + 1) * 512], po[nw][:], COPY)
                            else:
                                nc.vector.tensor_copy(
                                    ob[:, nw * 512:(nw + 1) * 512], po[nw][:])
                        nc.sync.dma_start(out[m * 128:(m + 1) * 128, :], ob[:])

    nc.finalize()
    return nc


def _get_plan_and_nc(sparse_mask):
    key = "nc"
    if key in _cache:
        return _cache[key]
    bm = np.asarray(sparse_mask)[::_BLK, ::_BLK]
    plan = _build_plan(bm)
    nc = _build_nc(plan)
    _cache[key] = (plan, nc)
    return plan, nc


def kernel(hidden_states, Wq, Wk, Wv, Wo, sparse_mask):
    from concourse.bass_utils import run_bass_kernel_spmd

    trace = bool(os.environ.get("BIGBIRD_TRACE"))
    if trace and "antenv.axon_hooks" not in sys.modules:
        try:
            import trn_agent_boot.trn_boot as _tb
            _hook = _tb._ntff_profile_via_ctypes("/opt/axon/libaxon_pjrt.so")
            _m = types.ModuleType("antenv.axon_hooks")
            _m.get_axon_ntff_profile_hook = lambda: _hook
            _m.set_axon_ntff_profile_hook = lambda h: None
            sys.modules["antenv.axon_hooks"] = _m
            import concourse.bass_utils as _bu
            _bu.upload_artifacts = lambda tmpdir: tmpdir
        except Exception as e:
            print(f"trace hook setup failed: {e}", file=sys.stderr)
            trace = False

    hs = np.asarray(hidden_states, np.float32)
    Wq = np.asarray(Wq, np.float32)
    Wk = np.asarray(Wk, np.float32)
    Wv = np.asarray(Wv, np.float32)
    Wo = np.asarray(Wo, np.float32)

    plan, nc = _get_plan_and_nc(sparse_mask)

    in_maps = []
    for core in range(_NCORES):
        b, hg = core // 4, core % 4
        hs_sl = slice(hg * 256, (hg + 1) * 256)
        in_maps.append({
            "xt": np.ascontiguousarray(hs[b].T).astype(np.float16),
            "wq": (np.ascontiguousarray(Wq[hs_sl].T) * (1.0 / 8.0)).astype(np.float16),
            "wk": np.ascontiguousarray(Wk[hs_sl].T).astype(np.float16),
            "wv": np.ascontiguousarray(Wv[hs_sl].T).astype(np.float16),
            "wo": np.ascontiguousarray(Wo[:, hs_sl].T).astype(np.float16),
        })

    res = run_bass_kernel_spmd(nc, in_maps, list(range(_NCORES)), trace=trace)
    if trace:
        print(f"HW exec time: {res.exec_time_ns} ns")
        _cache["exec_time_ns"] = res.exec_time_ns

    out = np.zeros((_B, _L, _D), np.float32)
    for core in range(_NCORES):
        out[core // 4] += res.results[core]["out"].astype(np.float32)
    return out


# revision 34
# speedup vs baseline: 1.3169x; 1.3169x over previous
"""BigBird block-sparse attention on 8 Trainium2 NeuronCores.

Sharding: core = (batch b, head-group hg): b = core//4, hg = core%4.
Each core computes, for its batch and its 4 heads, all in f16:
  qT/kT = (W{q,k}[hs] @ x.T)            [256, 2048]  (q pre-scaled by 1/8)
  v     = x @ Wv[hs].T                  [2048, 256]  (natural layout + ones col)
  Scores are computed per PAIR of key blocks (2g, 2g+1): one [64,128] f16
  stationary (kT pair) streams the union of the pair's kept q-runs, writing
  both blocks' transposed scores [128, cols] in a single matmul.
  expS  = exp(S.T) packed in PSUM fills, evicted to SBUF (f16), then
          multiplied by a static 0/1 hole mask (built on-chip once).
  outT  = [v_pair|1].T @ expS           [65, 2048], K=128 full-array matmuls
  attnT = outT[0:64] * (1/outT[64]) per head  -> [256, 2048] f16
  out  += attnT.T @ Wo[:, hs].T         [2048, 1024] f16 partial per head grp
Host gathers: out[b] = f32 sum over the 4 head-group cores of that batch.
"""

import os
import sys
import types

import numpy as np

_B, _L, _D = 2, 2048, 1024
_H, _HD, _BLK = 16, 64, 64
_NB = _L // _BLK  # 32
_NG = _NB // 2    # 16 key-block pairs
_NCORES = 8
_HPC = 4  # heads per core
_FILLW = 1024  # packed-psum fill width (2 PSUM banks, f32)

_cache = {}


# --------------------------------------------------------------------------
# host-side plan: derive the pair-block mask structure once
# --------------------------------------------------------------------------
def _build_plan(bm):
    """bm: [NB, NB] bool block mask (bm[q, j] = q-block attends key-block j).

    Key blocks are processed in pairs (2g, 2g+1); each piece is a matmul over
    the union of the pair's kept q-blocks, all 128 out partitions at once.

    Returns dict with:
      fills: list of fills; each a list of pieces
             dict(g, q0, n, off, g2, avs=[(a, nb, off)...])
      n_fills
      holes: list of (rlo, rhi, col0, ncols) absolute expS col ranges to zero
             (rlo/rhi = 0/64 for even half, 64/128 for odd half)
      av_flags: {(fi, g, a): (start, stop)} per outT PSUM bank
    """
    holes = []  # (parity, abs_col, ncols), merged later

    fills = [[]]
    cur = [0]

    def close_fill():
        if fills[-1]:
            fills.append([])
            cur[0] = 0

    def place(g, q0, n, is_g2, A, Bk):
        cols = n * _BLK
        if cur[0] + cols > _FILLW:
            close_fill()
        off = cur[0]
        cur[0] += cols
        fi = len(fills) - 1
        if is_g2:
            avs = [(0, 1, off), (31, 1, off + _BLK)]
            qs = (0, 31)
        else:
            avs = []
            a = q0
            while a < q0 + n:
                lim = min(q0 + n, ((a // 8) + 1) * 8)
                avs.append((a, lim - a, off + (a - q0) * _BLK))
                a = lim
            qs = range(q0, q0 + n)
        for idx, b in enumerate(qs):
            c0 = fi * _FILLW + off + idx * _BLK
            if b not in A:
                holes.append((0, c0, _BLK))
            if b not in Bk:
                holes.append((1, c0, _BLK))
        fills[-1].append(dict(g=g, q0=q0, n=n, off=off, avs=avs, g2=is_g2))

    for g in range(_NG):
        ja, jb = 2 * g, 2 * g + 1
        A = set(np.nonzero(bm[:, ja])[0].tolist())
        Bk = set(np.nonzero(bm[:, jb])[0].tolist())
        qs = sorted(A | Bk)
        runs = []
        s = p = qs[0]
        for x in qs[1:]:
            if x == p + 1:
                p = x
            else:
                runs.append((s, p))
                s = p = x
        runs.append((s, p))
        g2 = (len(runs) >= 2 and runs[0] == (0, 0) and runs[-1] == (31, 31))
        if g2:
            runs = runs[1:-1]
            # 2-block strided piece {0, 31}; keep within one PSUM bank
            if cur[0] % 512 > 512 - 128:
                cur[0] = (cur[0] // 512 + 1) * 512
                if cur[0] >= _FILLW:
                    close_fill()
            place(g, 0, 2, True, A, Bk)
        for (s, e) in runs:
            q = s
            n = e - s + 1
            while n > 0:
                room = (512 - cur[0] % 512) // _BLK
                if room == 0:
                    cur[0] = (cur[0] // 512 + 1) * 512
                    if cur[0] >= _FILLW:
                        close_fill()
                    room = 8
                take = min(n, room, 8)
                place(g, q, take, False, A, Bk)
                q += take
                n -= take
    if not fills[-1]:
        fills.pop()

    # AV start/stop flags per outT PSUM bank (512-col granularity)
    exec_order = [(fi, pc["g"], a)
                  for fi, fill in enumerate(fills)
                  for pc in fill for (a, nb, off) in pc["avs"]]
    av_flags = {}
    first_seen = set()
    last_piece = {}
    for key in exec_order:
        last_piece[key[2] // 8] = key
    for key in exec_order:
        bank = key[2] // 8
        av_flags[key] = (bank not in first_seen, last_piece[bank] == key)
        first_seen.add(bank)

    # merge adjacent hole runs per parity
    merged = []
    for par in (0, 1):
        runs = sorted((c, n) for (p, c, n) in holes if p == par)
        i = 0
        while i < len(runs):
            c0, n0 = runs[i]
            j = i + 1
            while j < len(runs) and runs[j][0] == c0 + n0:
                n0 += runs[j][1]
                j += 1
            merged.append((par * 64, par * 64 + 64, c0, n0))
            i = j

    return dict(fills=fills, n_fills=len(fills), av_flags=av_flags,
                holes=merged)


# --------------------------------------------------------------------------
# numpy simulator of the planned pipeline (used by test_plan.py)
# --------------------------------------------------------------------------
def _sim_plan(plan, q, k, v):
    """q, k, v: [L, 64] f32 (q pre-scaled by 1/8). Returns attn out [L, 64]."""
    nf = plan["n_fills"]
    expS = np.zeros((128, nf * _FILLW), np.float32)
    mask = np.ones((128, nf * _FILLW), np.float32)
    for (rlo, rhi, c0, n) in plan["holes"]:
        mask[rlo:rhi, c0:c0 + n] = 0.0
    outT = np.zeros((65, _L), np.float64)
    for fi, fill in enumerate(plan["fills"]):
        ps = np.zeros((128, _FILLW), np.float32)
        for pc in fill:
            g, q0, n, off = pc["g"], pc["q0"], pc["n"], pc["off"]
            kp = k[g * 128:(g + 1) * 128]  # pair of key blocks
            if pc["g2"]:
                qsel = np.concatenate([q[0:64], q[31 * 64:32 * 64]], axis=0)
            else:
                qsel = q[q0 * 64:(q0 + n) * 64]
            ps[:, off:off + qsel.shape[0]] = kp @ qsel.T
        expS[:, fi * _FILLW:(fi + 1) * _FILLW] = np.exp(ps)
    expS *= mask
    for fi, fill in enumerate(plan["fills"]):
        for pc in fill:
            g = pc["g"]
            vj = np.concatenate(
                [v[g * 128:(g + 1) * 128], np.ones((128, 1), np.float32)],
                axis=1)
            for (a, n, off) in pc["avs"]:
                e = expS[:, fi * _FILLW + off: fi * _FILLW + off + n * 64]
                outT[:, a * 64:(a + n) * 64] += vj.T @ e
    return (outT[0:64] / outT[64]).T


# --------------------------------------------------------------------------
# bass kernel build
# --------------------------------------------------------------------------
def _patch_ldw_opt():
    """Re-enable walrus's LDWEIGHTS dedup pass (concourse pins it off)."""
    if os.environ.get("BIGBIRD_LDW_OPT", "0") != "1":
        return
    import concourse.bass_utils as bu
    if getattr(bu, "_bigbird_ldw_patched", False):
        return
    orig = bu.run_command

    def run_command(cmd, *a, **k):
        cmd = [c.replace("--enable-ldw-opt=false", "--enable-ldw-opt=true")
               if isinstance(c, str) else c for c in cmd]
        return orig(cmd, *a, **k)

    bu.run_command = run_command
    bu._bigbird_ldw_patched = True


def _build_nc(plan):
    _patch_ldw_opt()
    import concourse.bacc as bacc
    import concourse.mybir as mybir
    from concourse.tile import TileContext

    f32r = mybir.dt.float32r
    f32 = mybir.dt.float32
    f16 = mybir.dt.float16
    EXP = mybir.ActivationFunctionType.Exp
    COPY = mybir.ActivationFunctionType.Copy

    NKC = _D // 128   # 8 contraction chunks
    NM = _L // 128    # 16 L tiles
    nf = plan["n_fills"]

    nc = bacc.Bacc(None, target_bir_lowering=False)

    xt = nc.dram_tensor("xt", [_D, _L], f16, kind="ExternalInput")
    wq = nc.dram_tensor("wq", [_D, 256], f16, kind="ExternalInput")
    wk = nc.dram_tensor("wk", [_D, 256], f16, kind="ExternalInput")
    wv = nc.dram_tensor("wv", [_D, 256], f16, kind="ExternalInput")
    wo = nc.dram_tensor("wo", [256, _D], f16, kind="ExternalInput")
    out = nc.dram_tensor("out", [_L, _D], f16, kind="ExternalOutput")

    with TileContext(nc) as tc:
        with tc.tile_pool(name="persist_sb", bufs=1) as psb:
            # ---- persistent SBUF ----
            wo_sb = [psb.tile([128, _D], f16, name=f"wo{c}", tag=f"wo{c}")
                     for c in range(2)]
            qT = [psb.tile([128, _L], f16, name=f"qT{c}", tag=f"qT{c}")
                  for c in range(2)]
            kT = [psb.tile([128, _L], f16, name=f"kT{c}", tag=f"kT{c}")
                  for c in range(2)]
            # v' packed: per head 16 pairs x 65 cols (64 v + ones)
            vp = psb.tile([128, _HPC * 16 * 65], f16, name="vp", tag="vp")
            attnT = [psb.tile([128, _L], f16, name=f"attnT{c}", tag=f"attnT{c}")
                     for c in range(2)]
            ones_sb = psb.tile([1, 64], f16, name="ones_sb", tag="ones_sb")
            mask_sb = psb.tile([128, nf * _FILLW], f16, name="mask_sb",
                               tag="mask_sb")
            nc.vector.memset(ones_sb[:], 1.0)
            for c in range(2):
                nc.sync.dma_start(wo_sb[c][:], wo[c * 128:(c + 1) * 128, :])
            # ones columns of v'
            for h in range(_HPC):
                nc.vector.memset(
                    vp[:, h * 1040 + 64: h * 1040 + 16 * 65: 65], 1.0)
            # static hole mask (shared by all heads), built during load phase
            nc.vector.memset(mask_sb[:], 1.0)
            for (rlo, rhi, c0, ncols) in plan["holes"]:
                nc.vector.memset(mask_sb[rlo:rhi, c0:c0 + ncols], 0.0)

            with tc.tile_pool(name="load_sb", bufs=1) as lsb:
                # ---- input DMA ----
                xt_sb = [lsb.tile([128, _L], f16, name=f"xt{kc}", tag=f"xt{kc}")
                         for kc in range(NKC)]
                wq_sb = [lsb.tile([128, 256], f16, name=f"wq{kc}", tag=f"wq{kc}")
                         for kc in range(NKC)]
                wk_sb = [lsb.tile([128, 256], f16, name=f"wk{kc}", tag=f"wk{kc}")
                         for kc in range(NKC)]
                wv_sb = [lsb.tile([128, 256], f16, name=f"wv{kc}", tag=f"wv{kc}")
                         for kc in range(NKC)]
                for kc in range(NKC):
                    nc.sync.dma_start(wq_sb[kc][:], wq[kc * 128:(kc + 1) * 128, :])
                    nc.sync.dma_start(wk_sb[kc][:], wk[kc * 128:(kc + 1) * 128, :])
                    nc.sync.dma_start(xt_sb[kc][:], xt[kc * 128:(kc + 1) * 128, :])
                    nc.sync.dma_start(wv_sb[kc][:], wv[kc * 128:(kc + 1) * 128, :])

                # ---- projections (Q, K) ----
                with tc.tile_pool(name="proj_ps", bufs=1, space="PSUM") as pps:
                    for (w_sb, dst) in ((wq_sb, qT), (wk_sb, kT)):
                        for half in range(2):  # L halves for earlier PE start
                            pt = [pps.tile([128, 512], f32, name=f"pp{mc}{nwi}",
                                           tag=f"pp{mc}{nwi}")
                                  for mc in range(2) for nwi in range(2)]
                            for kc in range(NKC):
                                for mc in range(2):
                                    for nwi in range(2):
                                        nw = half * 2 + nwi
                                        nc.tensor.matmul(
                                            pt[mc * 2 + nwi][:],
                                            w_sb[kc][:, mc * 128:(mc + 1) * 128],
                                            xt_sb[kc][:, nw * 512:(nw + 1) * 512],
                                            start=(kc == 0), stop=(kc == NKC - 1))
                            for mc in range(2):
                                for nwi in range(2):
                                    nw = half * 2 + nwi
                                    if nwi == 0:
                                        nc.scalar.activation(
                                            dst[mc][:, nw * 512:(nw + 1) * 512],
                                            pt[mc * 2 + nwi][:], COPY)
                                    else:
                                        nc.vector.tensor_copy(
                                            dst[mc][:, nw * 512:(nw + 1) * 512],
                                            pt[mc * 2 + nwi][:])
                    # ---- V projection (natural layout) ----
                    for m in range(NM):
                        pv = pps.tile([128, 256], f32, name="pv", tag="pv", bufs=3)
                        for kc in range(NKC):
                            nc.tensor.matmul(
                                pv[:],
                                xt_sb[kc][:, m * 128:(m + 1) * 128],
                                wv_sb[kc][:],
                                start=(kc == 0), stop=(kc == NKC - 1))
                        # scatter 4 heads into v' tile (pair index = m)
                        vdst = vp[:].rearrange("p (h c) -> p h c", c=1040)
                        vsrc = pv[:].rearrange("p (h d) -> p h d", d=64)
                        nc.vector.tensor_copy(
                            vdst[:, :, m * 65: m * 65 + 64], vsrc[:, :, :])

            with tc.tile_pool(name="att_sb", bufs=1) as asb:
                # ---- attention per head ----
                with tc.tile_pool(name="att_ps", bufs=1, space="PSUM") as aps:
                    for h in range(_HPC):
                        c, pb = h // 2, (h % 2) * 64
                        expS = asb.tile([128, nf * _FILLW], f16, name="expS",
                                        tag="expS", bufs=2)
                        outT = aps.tile([128, _L], f32, name="outT", tag="outT")
                        # rows 0-63: attn out, row 64: sums
                        oT_sb = asb.tile([65, _L], f32r, name="oT_sb",
                                         tag="oT_sb", bufs=2)
                        rec = asb.tile([1, _L], f16, name="rec",
                                       tag="rec", bufs=2)
                        for fi, fill in enumerate(plan["fills"]):
                            ps = aps.tile([128, _FILLW], f32, name="sfill",
                                          tag="sfill", bufs=2)
                            for pc in fill:
                                g, q0, n, off = pc["g"], pc["q0"], pc["n"], pc["off"]
                                if pc["g2"]:
                                    rhs = qT[c][pb:pb + 64, :].rearrange(
                                        "p (a b) -> p a b", b=64)[:, 0:32:31, :]
                                else:
                                    rhs = qT[c][pb:pb + 64, q0 * 64:(q0 + n) * 64]
                                nc.tensor.matmul(
                                    ps[:, off:off + n * 64],
                                    kT[c][pb:pb + 64, g * 128:(g + 1) * 128],
                                    rhs,
                                    start=True, stop=True,
                                    tile_position=(pb, 0))
                            fsl = slice(fi * _FILLW, (fi + 1) * _FILLW)
                            nc.scalar.activation(expS[:, fsl], ps[:], EXP)
                            nc.vector.tensor_mul(
                                expS[:, fsl], expS[:, fsl], mask_sb[:, fsl])
                            for pc in fill:
                                g = pc["g"]
                                for (a, n, off) in pc["avs"]:
                                    st, sp = plan["av_flags"][(fi, g, a)]
                                    nc.tensor.matmul(
                                        outT[0:65, a * 64:(a + n) * 64],
                                        vp[:, h * 1040 + g * 65:
                                           h * 1040 + g * 65 + 65],
                                        expS[:, fi * _FILLW + off:
                                             fi * _FILLW + off + n * 64],
                                        start=st, stop=sp,
                                        tile_position=(0, 0))
                        # eviction + normalization
                        nc.scalar.activation(oT_sb[0:65, :], outT[0:65, :], COPY)
                        # reciprocal on one partition is ~13us; reshape the sums
                        # row across 128 partitions via SBUF->SBUF DMA instead
                        recT = asb.tile([128, 16], f32r, name="recT",
                                        tag="recT", bufs=2)
                        recT2 = asb.tile([128, 16], f16, name="recT2",
                                         tag="recT2", bufs=2)
                        nc.sync.dma_start(recT[:], oT_sb[64:65, :])
                        with nc.allow_low_precision("fp16 softmax denominators"):
                            nc.vector.reciprocal(recT2[:], recT[:])
                        nc.sync.dma_start(rec[:], recT2[:])
                        # broadcast 1/sums back into the (already evicted)
                        # outT rows 0-63
                        for w in range(4):
                            nc.tensor.matmul(
                                outT[0:64, w * 512:(w + 1) * 512],
                                ones_sb[:],
                                rec[:, w * 512:(w + 1) * 512],
                                start=True, stop=True)
                        for w in range(4):
                            nc.vector.tensor_mul(
                                attnT[c][pb:pb + 64, w * 512:(w + 1) * 512],
                                oT_sb[0:64, w * 512:(w + 1) * 512],
                                outT[0:64, w * 512:(w + 1) * 512])

                # ---- output projection ----
                with tc.tile_pool(name="o_ps", bufs=4, space="PSUM") as ops:
                    for m in range(NM):
                        po = [ops.tile([128, 512], f32, name="po", tag=f"po{nw}")
                              for nw in range(2)]
                        for nw in range(2):
                            for c in range(2):
                                nc.tensor.matmul(
                                    po[nw][:],
                                    attnT[c][:, m * 128:(m + 1) * 128],
                                    wo_sb[c][:, nw * 512:(nw + 1) * 512],
                                    start=(c == 0), stop=(c == 1))
                        ob = asb.tile([128, _D], f16, name="ob", tag="ob", bufs=3)
                        for nw in range(2):
                            if nw == 0:
                                nc.scalar.activation(
                                    ob[:, nw * 512:(nw + 1) * 512], po[nw][:], COPY)
                            else:
                                nc.vector.tensor_copy(
                                    ob[:, nw * 512:(nw + 1) * 512], po[nw][:])
                        nc.sync.dma_start(out[m * 128:(m + 1) * 128, :], ob[:])

    nc.finalize()
    return nc


def _get_plan_and_nc(sparse_mask):
    key = "nc"
    if key in _cache:
        return _cache[key]
    bm = np.asarray(sparse_mask)[::_BLK, ::_BLK]
    plan = _build_plan(bm)
    nc = _build_nc(plan)
    _cache[key] = (plan, nc)
    return plan, nc


def kernel(hidden_states, Wq, Wk, Wv, Wo, sparse_mask):
    from concourse.bass_utils import run_bass_kernel_spmd

    trace = bool(os.environ.get("BIGBIRD_TRACE"))
    if trace and "antenv.axon_hooks" not in sys.modules:
        try:
            import trn_agent_boot.trn_boot as _tb
            _hook = _tb._ntff_profile_via_ctypes("/opt/axon/libaxon_pjrt.so")
            _m = types.ModuleType("antenv.axon_hooks")
            _m.get_axon_ntff_profile_hook = lambda: _hook
            _m.set_axon_ntff_profile_hook = lambda h: None
            sys.modules["antenv.axon_hooks"] = _m
            import concourse.bass_utils as _bu
            _bu.upload_artifacts = lambda tmpdir: tmpdir
        except Exception as e:
            print(f"trace hook setup failed: {e}", file=sys.stderr)
            trace = False

    hs = np.asarray(hidden_states, np.float32)
    Wq = np.asarray(Wq, np.float32)
    Wk = np.asarray(Wk, np.float32)
    Wv = np.asarray(Wv, np.float32)
    Wo = np.asarray(Wo, np.float32)

    plan, nc = _get_plan_and_nc(sparse_mask)

    in_maps = []
    for core in range(_NCORES):
        b, hg = core // 4, core % 4
        hs_sl = slice(hg * 256, (hg + 1) * 256)
        in_maps.append({
            "xt": np.ascontiguousarray(hs[b].T).astype(np.float16),
            "wq": (np.ascontiguousarray(Wq[hs_sl].T) * (1.0 / 8.0)).astype(np.float16),
            "wk": np.ascontiguousarray(Wk[hs_sl].T).astype(np.float16),
            "wv": np.ascontiguousarray(Wv[hs_sl].T).astype(np.float16),
            "wo": np.ascontiguousarray(Wo[:, hs_sl].T).astype(np.float16),
        })

    res = run_bass_kernel_spmd(nc, in_maps, list(range(_NCORES)), trace=trace)
    if trace:
        print(f"HW exec time: {res.exec_time_ns} ns")
        _cache["exec_time_ns"] = res.exec_time_ns

    out = np.zeros((_B, _L, _D), np.float32)
    for core in range(_NCORES):
        out[core // 4] += res.results[core]["out"].astype(np.float32)
    return out
